# revision 57
# baseline (speedup 1.0000x reference)
"""Trainium2 Bass kernel for nn_FCond (FiLM-conditioned MLP chain).

Reference computation (B=32, N=100000, D=3, CDIM=128):
    h = x
    for kblk in [0, 1, 2, 2, 2, 2]:
        h = tanh((h @ Wk.T + bk) * sigmoid(c @ Wsk.T + bsk) + (c @ Wbk.T + bbk))

Since the FiLM conditioning depends only on (c, weights), each (batch,
block) reduces to an affine map  h' = tanh(A_kb @ h + d_kb)  with
A_kb [3,3], d_kb [3] precomputed on the host in float64.

Device strategy (pure data parallel over 8 cores, 4 batches/core):
  - Layout: 42 point-streams of 3 comps = 126 partitions, plus a
    constant-1.0 row (126) and a zero row (127). Each batch owns 10
    full streams; each batch PAIR shares one boundary stream that
    switches batch at column BSPLIT (handled by a second weight zone
    and a matmul split at that column).
  - Each block applies a block-diagonal [128x128] fp16 matmul on
    TensorE (PSUM f32), then a nonlinearity evacuating PSUM->SBUF fp16.
  - TWO nonlinearity lanes split the per-block columns (the previous
    all-ACT version was tanh-bound at ~74.4us):
      * ACT lane: 6 chunks x ~1353 cols/block, exact tanh on ScalarE
        (1 col/cyc @ 1.2 GHz + ~310cyc/instr overhead).
      * DVE lane: 2 chunks x 678 cols/block on the otherwise-idle
        VectorE: clamped odd deg-9 minimax polynomial
        y = x*c4(t^2+a1 t+b1)(t^2+a2 t+b2), t = x^2, as 10 DVE
        instrs/chunk using ONLY tensor_scalar (4x mode) and
        tensor_tensor (2x) -- scalar_tensor_tensor has no 2x uop and
        measures 1x. ~4.7 ns/col vs ACT 0.83, so the split is ~14.2%.
        The clamp bound U is the fp16 input where the full fp16 chain
        evaluates to exactly 1.0, so the ones-row (which carries the
        affine bias through the matmul via W[126,col]=d and
        W[126,126]=16) regenerates exactly, like tanh(16)==1.0 does on
        the ACT lane. End-to-end rel err 1.12e-2 (tol 2e-2; the
        deg-11 variant gives 4.9e-3 at ~+1us).
  - PSUM regions are BANK-DISJOINT -- a DVE read of a PSUM bank that
    ACT/PE concurrently touch crashes the exec unit (found by
    bisection; ACT+PE sharing a bank is fine): ACT ping-pong
    [0:1536]/[1536:3072] (banks 0-2/3-5), DVE single slot [3072:...]
    (banks 6-7). Matmul windows split at 512-col bank boundaries and
    BSPLIT. The DVE slot is single-buffered: PE's next V matmul waits
    only for the previous chunk's evac (its first DVE op).
  - Hand-scheduled engine programs: ACT stages rotate in groups
    (0,1)/(2,3)/(4,5) over blocks, DVE stages rotate (V0,V1); PE
    serves both lanes ordered by consumer deadline, with one folded
    act_sem/dve_sem wait per stage covering input-ready + PSUM WAR.
    dve_sem increments only on each DVE stage's evac + final op
    (dense per-op then_inc risks the cayman event-accel deadlock).
  - DMA: weights FIRST (in the all-ACT version the first matmul
    stalled ~3.5us behind x0 on the shared hw queues), then an
    872-col first piece of chunk 0 (split PE/ACT stage 0) so the
    first tanh starts ~11.3us, xV0 early on the scalar queue for the
    V-lane start, bulk transfers + weight set 2 gated out of the
    fill window,
    software-DGE (gpsimd-queued) transfers on their own semaphores
    (their completion order is undefined and CoreSim requires
    sem-start-at-0). Outputs stream per chunk as each last-block
    chunk lands; the final ACT tanh and final DVE chunk are
    column-split so their DMA starts early.

Numerics: weights/bias/activations fp16 (PE @ 1 col/cyc @ 2.4 GHz),
PSUM f32, ACT tanh exact, DVE polynomial max err ~1.2e-2
pre-attenuation. Measured end-to-end rel err 1.12e-2 (tolerance 2e-2).
Measured HW exec: ~67.0-68.2us (all-ACT predecessor 74.4us, original
baseline 78.6us). ACT runs 11.3->60.3us and DVE 13.4->60.3us, both
near-saturated; ~5.5us fixed preamble and ~5.5us output-DMA + barrier
+ NEFF-finalize tail.
"""
import sys
import types

import numpy as np

B, N, D, CDIM = 32, 100000, 3, 128
NCORES = 8
BPC = B // NCORES          # batches per core
P = 128                    # partitions

NST_R = 42                 # streams per core
LC = 9524                  # columns (points per stream)
BSPLIT = 4760              # boundary-stream batch-switch column
NFULL = 95240              # points covered by the 10 full streams

# ---- chunk grid: 6 ACT chunks + 2 DVE chunks per block ----
# PSUM regions must be BANK-DISJOINT: a DVE read of a PSUM bank that
# ACT/PE concurrently touch crashes the exec unit. PSA0 = banks 0-2,
# PSA1 = banks 3-5, PSV = banks 6-7. Even-sized ACT rotation groups fix
# each chunk's psum parity, so even chunks <= 1536 live in PSA0 and odd
# ones in PSA1.
CAS = [1362, 1362, 1362, 1362, 1360, 1360]     # ACT chunk sizes
CV = 678                                       # DVE chunk size
SP0 = 872                  # block-0 chunk-0 split for a fast pipeline fill
NA, NV = 6, 2
A_OFF = [sum(CAS[:i]) for i in range(NA)]
AV = sum(CAS)
V_OFF = [AV + j * CV for j in range(NV)]       # 8008, 8766
PSA_BASE = (0, 1536)       # ACT psum ping-pong bases (banks 0-2 / 3-5)
PSV_BASE = 3072            # DVE psum slot base (banks 6-7)
A_GROUPS = ((0, 1), (2, 3), (4, 5))
WSETS = (0, 1, 2, 2, 2, 2)
TH = (CAS[-1] // 2 + 3) & ~3   # last ACT tanh column split (668)
VH = (CV // 2 + 3) & ~3        # last DVE chunk column split (380)

# ---- deg-9 odd minimax polynomial for tanh on [0,3.25], constrained
# p(3.34)=1 and scaled so the fp16 chain crosses 1.0; factored for
# TS+TT evaluation. U is the fp16 clamp input where the fp16 chain
# yields exactly 1.0 (ones-row trick).
PC4 = 0.00010215534319844812 * 1.0005
PA1 = -26.393814652081367
PB1 = 195.05314722643175
PA2 = -5.005435676115361
PB2 = 48.33620445619426
PU = 3.33203125

PROFILE = False            # set by test harness; collects HW exec time
LAST_EXEC_NS = None

_CACHE = {}


def _install_profile_shim():
    """Register the NTFF profile hook (missing antenv.axon_hooks in this
    container) so run_bass_kernel_spmd(trace=True) can report exec time."""
    if "antenv.axon_hooks" in sys.modules:
        return
    mod = types.ModuleType("antenv.axon_hooks")
    _state = {"hook": None}
    mod.set_axon_ntff_profile_hook = lambda h: _state.__setitem__("hook", h)
    mod.get_axon_ntff_profile_hook = lambda: _state["hook"]
    sys.modules["antenv.axon_hooks"] = mod
    try:
        from trn_agent_boot.trn_boot import _ntff_profile_via_ctypes
        mod.set_axon_ntff_profile_hook(
            _ntff_profile_via_ctypes("/opt/axon/libaxon_pjrt.so"))
    except Exception:
        pass
    import concourse.bass_utils as bu
    bu.upload_artifacts = lambda tmpdir: f"local:{tmpdir}"


def _mm_windows(col0, cols, ps_base):
    """Matmul windows (local lo, local hi, zone): split at PSUM 512-col
    bank boundaries (psum coords) and at the BSPLIT zone switch."""
    cuts = {0, cols}
    b = (ps_base // 512 + 1) * 512
    while b < ps_base + cols:
        cuts.add(b - ps_base)
        b += 512
    if col0 < BSPLIT < col0 + cols:
        cuts.add(BSPLIT - col0)
    cs = sorted(cuts)
    return [(lo, hi, 0 if (col0 + lo) < BSPLIT else 1)
            for lo, hi in zip(cs, cs[1:])]


def _build_program():
    import concourse.bass as bass
    import concourse.bacc as bacc
    from concourse import mybir

    f32 = mybir.dt.float32
    f16 = mybir.dt.float16
    Tanh = mybir.ActivationFunctionType.Tanh
    Alu = mybir.AluOpType

    nc = bacc.Bacc("TRN2", target_bir_lowering=False, debug=False)
    x_d = nc.declare_dram_parameter("x", [P, LC], f16, isOutput=False)
    w_d = nc.declare_dram_parameter("w", [P, 6 * P], f16, isOutput=False)
    y_d = nc.declare_dram_parameter("y", [P, LC], f16, isOutput=True)

    NCH = NA + NV                       # 8 chunks: 0-5 ACT, 6-7 DVE
    C_OFF = A_OFF + V_OFF
    C_LEN = CAS + [CV] * NV

    # static SBUF tensors. ha/hb A-chunk tensors are padded to width
    # 1810: every fast (182cyc-overhead) tanh ever measured wrote into
    # a 1810-wide dest; these tensors are never DMA'd so the strided
    # AP cannot hit the slow strided-DMA path.
    HPAD = 1810
    def _alloc_hab(nm, c):
        if c < NA:
            return nc.alloc_sbuf_tensor(nm, [P, HPAD],
                                        f16).ap()[:, 0:C_LEN[c]]
        return nc.alloc_sbuf_tensor(nm, [P, C_LEN[c]], f16).ap()
    xin = [nc.alloc_sbuf_tensor(f"xin{c}", [P, C_LEN[c]], f16).ap()
           for c in range(NCH)]
    ha = [_alloc_hab(f"ha{c}", c) for c in range(NCH)]
    hb = [_alloc_hab(f"hb{c}", c) for c in range(NCH)]
    yout = [nc.alloc_sbuf_tensor(f"yout{c}", [P, C_LEN[c]], f16).ap()
            for c in range(NCH)]
    wall = nc.alloc_sbuf_tensor("wall", [P, 6 * P], f16).ap()
    ps = nc.alloc_psum_tensor("ps", [P, 4096], f32).ap()
    # DVE polynomial temporaries
    vt = {n: nc.alloc_sbuf_tensor(f"vt_{n}", [P, CV], f16).ap()
          for n in ("xe", "t", "w", "u1", "u2", "f", "p", "p2")}

    # ---- stage tables ----
    a_stages = [(k, ci) for grp in A_GROUPS for k in range(6) for ci in grp]
    v_stages = [(k, j) for k in range(6) for j in range(NV)]
    idx_a = {kc: s for s, kc in enumerate(a_stages)}
    idx_v = {kc: s for s, kc in enumerate(v_stages)}
    NSA, NSV = len(a_stages), len(v_stages)

    # ACT instruction counts (stage 0 and the last stage are split)
    a_n = [2 if s in (0, NSA - 1) else 1 for s in range(NSA)]
    a_cum = np.cumsum(a_n).tolist()          # act_sem value after stage s
    # dve_sem incs per stage: evac + final y only (sparse then_inc: dense
    # per-op incs can trip the cayman event-accel deadlock); last stage's
    # final mult is split in two, each inc'ing
    v_n = [3 if s == NSV - 1 else 2 for s in range(NSV)]
    v_cum = np.cumsum(v_n).tolist()          # dve_sem value after stage sv

    # PE order: merge lanes by estimated consumer deadline
    EST_A, EST_V = 1408.0, 4107.0
    # A-stage 0 is split into two PE stages (Aa/Ab) for the fill
    ev = ([("Aa", 0, 0.0), ("Ab", 0, 730.0)]
          + [("A", s, s * EST_A) for s in range(1, NSA)]
          + [("V", sv, sv * EST_V + 500.0) for sv in range(NSV)])
    pe_order = sorted(ev, key=lambda e: e[2])
    pe_pos = {(lane, i): p for p, (lane, i, _) in enumerate(pe_order)}

    def h_in(k, c):
        if k == 0:
            return xin[c]
        return ha[c] if k % 2 == 1 else hb[c]

    def h_out(k, c):
        if k == 5:
            return yout[c]
        return ha[c] if k % 2 == 0 else hb[c]

    H = P // 2

    with (
        nc.Block(no_gpsimd_drain=True) as block,
        nc.semaphore("dxa0") as dxa0,
        nc.semaphore("dxa1") as dxa1,
        nc.semaphore("dxa2") as dxa2,
        nc.semaphore("dxa3") as dxa3,
        nc.semaphore("dxa4") as dxa4,
        nc.semaphore("dxa5") as dxa5,
        nc.semaphore("dxa0a") as dxa0a,
        nc.semaphore("dxv0") as dxv0,
        nc.semaphore("dxv1") as dxv1,
        nc.semaphore("dww") as dww,
        nc.semaphore("dww12") as dww12,
        nc.semaphore("dout") as dout,
        nc.semaphore("dxg") as dxg,
        nc.semaphore("doutg") as doutg,
        nc.semaphore("act_sem") as act_sem,
        nc.semaphore("pe_sem") as pe_sem,
        nc.semaphore("dve_sem") as dve_sem,
    ):
        dxa = [dxa0, dxa1, dxa2, dxa3, dxa4, dxa5]
        dxv = [dxv0, dxv1]

        def x_dma(eng, c, p0, p1, sem):
            eng.dma_start(out=xin[c][p0:p1, :],
                          in_=x_d[p0:p1, C_OFF[c]:C_OFF[c] + C_LEN[c]]
                          ).then_inc(sem, 16)

        def y_dma(eng, c, sem, req, p0=0, p1=P, c0=0, c1=None, dsem=None):
            c1 = C_LEN[c] if c1 is None else c1
            eng.dma_start(out=y_d[p0:p1, C_OFF[c] + c0:C_OFF[c] + c1],
                          in_=yout[c][p0:p1, c0:c1]
                          )._wait_ge(sem, req).then_inc(dsem or dout, 16)

        N_OUT = 6   # output transfers on the sync/scalar queues
        N_OUTG = 7  # output transfers on the gpsimd (software-DGE) queue

        @block.sync
        def _(sync: bass.BassEngine):
            # weights FIRST: the first matmul needs w; in the baseline w
            # queued behind the big x0 transfer on the shared hw queues
            # and the fill cost ~9us.
            sync.dma_start(out=wall[0:H, 0:2 * P], in_=w_d[0:H, 0:2 * P]
                           ).then_inc(dww, 16)
            # small first piece of chunk 0: the first matmul+tanh only
            # need [0:512], which lands ~1.5us before the full chunk
            sync.dma_start(out=xin[0][0:H, 0:SP0], in_=x_d[0:H, 0:SP0]
                           ).then_inc(dxa0a, 16)
            sync.dma_start(out=xin[0][0:H, SP0:C_LEN[0]],
                           in_=x_d[0:H, SP0:C_LEN[0]]).then_inc(dxa0, 16)
            x_dma(sync, 1, 0, H, dxa1)
            # weight set 1 now; set 2 (first needed by block 2, ~19us)
            # moves out of the fill window. Both ride the sync queue so
            # their completion order matches issue order.
            sync.dma_start(out=wall[:, 2 * P:4 * P],
                           in_=w_d[:, 2 * P:4 * P]).then_inc(dww12, 16)
            # keep bulk transfers out of the fill window
            sync.wait_ge(dxa1, 32)
            sync.dma_start(out=wall[:, 4 * P:6 * P],
                           in_=w_d[:, 4 * P:6 * P]).then_inc(dww12, 16)
            x_dma(sync, NA + 1, 0, P, dxv1)     # xV1 whole
            x_dma(sync, 2, 0, H, dxa2)
            x_dma(sync, 3, 0, H, dxa3)
            x_dma(sync, 4, 0, H, dxa4)
            x_dma(sync, 5, 0, H, dxa5)
            # output tail (in readiness order; queue is in-order)
            y_dma(sync, 3, act_sem, a_cum[idx_a[(5, 3)]])
            y_dma(sync, 4, act_sem, a_cum[idx_a[(5, 4)]], 0, H)
            y_dma(sync, 5, act_sem, a_cum[NSA - 1] - 1, 0, H, 0, TH)
            y_dma(sync, 5, act_sem, a_cum[NSA - 1], 0, H, TH, CAS[-1])
            sync.wait_ge(dout, 16 * N_OUT)
            sync.wait_ge(doutg, 16 * N_OUTG)

        @block.scalar
        def _(scalar: bass.BassEngine):
            scalar.dma_start(out=wall[H:P, 0:2 * P], in_=w_d[H:P, 0:2 * P]
                             ).then_inc(dww, 16)
            scalar.dma_start(out=xin[0][H:P, 0:SP0], in_=x_d[H:P, 0:SP0]
                             ).then_inc(dxa0a, 16)
            scalar.dma_start(out=xin[0][H:P, SP0:C_LEN[0]],
                             in_=x_d[H:P, SP0:C_LEN[0]]).then_inc(dxa0, 16)
            x_dma(scalar, NA, 0, P, dxv0)       # xV0 whole
            x_dma(scalar, 1, H, P, dxa1)
            for s, (k, ci) in enumerate(a_stages):
                base = PSA_BASE[s % 2]
                cl = C_LEN[ci]
                if s == 0:
                    for (a, b_), key in (((0, SP0), "Aa"), ((SP0, cl),
                                                           "Ab")):
                        scalar.activation(
                            h_out(k, ci)[:, a:b_], ps[:, base + a:base + b_],
                            Tanh, bias=0.0, scale=1.0,
                        )._wait_ge(pe_sem, pe_pos[(key, 0)] + 1
                                   ).then_inc(act_sem, 1)
                elif s == NSA - 1:
                    for a, b_ in ((0, TH), (TH, cl)):
                        act = scalar.activation(
                            h_out(k, ci)[:, a:b_], ps[:, base + a:base + b_],
                            Tanh, bias=0.0, scale=1.0)
                        if a == 0:
                            act._wait_ge(pe_sem, pe_pos[("A", s)] + 1)
                        act.then_inc(act_sem, 1)
                else:
                    scalar.activation(
                        h_out(k, ci), ps[:, base:base + cl], Tanh,
                        bias=0.0, scale=1.0,
                    )._wait_ge(pe_sem, pe_pos[("A", s)] + 1
                               ).then_inc(act_sem, 1)
            # remaining output quarters on the now-idle ACT queue
            y_dma(scalar, 5, act_sem, a_cum[NSA - 1], H, P, TH, CAS[-1])
            y_dma(scalar, NA + 1, dve_sem, v_cum[NSV - 1], 0, P, VH, CV)

        @block.gpsimd
        def _(g: bass.BassEngine):
            # stay clear of the fill window; software-DGE transfers use
            # their own semaphores (dxg cumulative in queue order)
            g.wait_ge(dxa1, 32)
            x_dma(g, 2, H, P, dxg)
            x_dma(g, 3, H, P, dxg)
            x_dma(g, 4, H, P, dxg)
            x_dma(g, 5, H, P, dxg)
            y_dma(g, 0, act_sem, a_cum[idx_a[(5, 0)]], dsem=doutg)
            y_dma(g, 1, act_sem, a_cum[idx_a[(5, 1)]], dsem=doutg)
            y_dma(g, 2, act_sem, a_cum[idx_a[(5, 2)]], dsem=doutg)
            y_dma(g, NA, dve_sem, v_cum[idx_v[(5, 0)]], dsem=doutg)
            y_dma(g, 4, act_sem, a_cum[idx_a[(5, 4)]], H, P, dsem=doutg)
            y_dma(g, 5, act_sem, a_cum[NSA - 1] - 1, H, P, 0, TH,
                  dsem=doutg)
            y_dma(g, NA + 1, dve_sem, v_cum[NSV - 1] - 1, 0, P, 0, VH,
                  dsem=doutg)

        def _vector_body(v: bass.BassEngine):
            psv = ps[:, PSV_BASE:PSV_BASE + CV]
            xe, t, w, u1, u2, f, p, p2 = (vt[n] for n in
                                          ("xe", "t", "w", "u1", "u2",
                                           "f", "p", "p2"))
            for sv, (k, j) in enumerate(v_stages):
                v.wait_ge(pe_sem, pe_pos[("V", sv)] + 1)
                # evac + clamp from PSUM f32 -> SBUF fp16; within-engine
                # deps below ride program order (the DVE pipe drains
                # between ops), so only the evac and the final y inc
                # dve_sem for PE / DMA consumers
                v.tensor_scalar(xe, psv, PU, -PU, Alu.min, Alu.max
                                ).then_inc(dve_sem, 1)
                # factored deg-9: all TT (2x) / TS (4x); STT would
                # run at 1x (no 2x uop) so each (a+s)*b is TS+TT
                v.tensor_tensor(t, xe, xe, Alu.mult)
                v.tensor_scalar(w, t, PA1, None, Alu.add)
                v.tensor_tensor(u1, w, t, Alu.mult)
                v.tensor_scalar(w, t, PA2, None, Alu.add)
                v.tensor_tensor(u2, w, t, Alu.mult)
                v.tensor_scalar(f, u1, PB1, PC4, Alu.add, Alu.mult)
                v.tensor_scalar(w, u2, PB2, None, Alu.add)
                v.tensor_tensor(p2, w, f, Alu.mult)
                out = h_out(k, NA + j)
                if sv == NSV - 1:
                    v.tensor_tensor(out[:, 0:VH], p2[:, 0:VH], xe[:, 0:VH],
                                    Alu.mult).then_inc(dve_sem, 1)
                    v.tensor_tensor(out[:, VH:CV], p2[:, VH:CV],
                                    xe[:, VH:CV], Alu.mult
                                    ).then_inc(dve_sem, 1)
                else:
                    v.tensor_tensor(out, p2, xe, Alu.mult
                                    ).then_inc(dve_sem, 1)

        block.vector(_vector_body)

        @block.tensor
        def _(tensor: bass.BassEngine):
            tensor.wait_ge(dww, 32)
            seen1 = seen2 = False
            for lane, i, _t in pe_order:
                if lane in ("A", "Aa", "Ab"):
                    s = i
                    k, ci = a_stages[s]
                    base = PSA_BASE[s % 2]
                    coff, clen = A_OFF[ci], C_LEN[ci]
                    if lane == "Aa":
                        clen = SP0
                    elif lane == "Ab":
                        base, coff = base + SP0, coff + SP0
                        clen = C_LEN[ci] - SP0
                    rhs = h_in(k, ci)
                    if lane == "Ab":
                        rhs = rhs[:, SP0:]
                else:
                    sv = i
                    k, j = v_stages[sv]
                    base = PSV_BASE
                    coff, clen = V_OFF[j], CV
                    rhs = h_in(k, NA + j)
                ks = WSETS[k]
                if ks >= 1 and not seen1:
                    tensor.wait_ge(dww12, 16)   # set 1
                    seen1 = True
                if ks == 2 and not seen2:
                    tensor.wait_ge(dww12, 32)   # + set 2 (same queue)
                    seen2 = True
                if k == 0:
                    if lane != "V":
                        if lane == "Aa":
                            tensor.wait_ge(dxa0a, 32)
                        elif lane == "Ab":
                            tensor.wait_ge(dxa[0], 32)
                        elif ci < 2 and s > 0:
                            tensor.wait_ge(dxa[ci], 32)
                        elif ci >= 2:
                            # top half on sync; bottom halves ride the
                            # gpsimd software-DGE queue whose completion
                            # order is not guaranteed -> wait for all 4
                            # (they land ~16us; first needed ~26us)
                            tensor.wait_ge(dxa[ci], 16)
                            tensor.wait_ge(dxg, 64)
                    else:
                        tensor.wait_ge(dxv[j], 16)
                # folded input-ready + PSUM WAR wait
                wv = 0
                if lane in ("A", "Aa", "Ab"):
                    if s >= 2:
                        wv = a_cum[s - 2]
                    if k > 0:
                        wv = max(wv, a_cum[idx_a[(k - 1, ci)]])
                else:
                    if sv >= 1:
                        wv = (v_cum[sv - 2] + 1) if sv >= 2 else 1
                wins = _mm_windows(coff, clen, base)
                for wi, (lo, hi, zone) in enumerate(wins):
                    kz = ks * 2 + zone
                    mm = tensor.matmul(ps[:, base + lo:base + hi],
                                       wall[:, kz * P:(kz + 1) * P],
                                       rhs[:, lo:hi],
                                       start=True, stop=True)
                    if wi == 0 and wv > 0:
                        if lane == "V":
                            mm._wait_ge(dve_sem, wv)
                        else:
                            mm._wait_ge(act_sem, wv)
                mm.then_inc(pe_sem, 1)

    nc.compile()
    return nc


def _film_params(c, Wk, bk, Wsk, bsk, Wbk, bbk):
    """A[b] = diag(scale[b]) @ Wk ; d[b] = scale[b]*bk + shift[b], float64."""
    c = c.astype(np.float64)
    scale = 1.0 / (1.0 + np.exp(-(c @ Wsk.astype(np.float64).T
                                  + bsk.astype(np.float64))))     # [B,3]
    shift = c @ Wbk.astype(np.float64).T + bbk.astype(np.float64)  # [B,3]
    A = scale[:, :, None] * Wk.astype(np.float64)[None]            # [B,3,3]
    d = scale * bk.astype(np.float64) + shift                      # [B,3]
    return A, d


def kernel(t, x, c,
           W0, b0, Ws0, bs0, Wb0, bb0,
           W1, b1, Ws1, bs1, Wb1, bb1,
           W2, b2, Ws2, bs2, Wb2, bb2):
    global LAST_EXEC_NS
    if PROFILE:
        _install_profile_shim()
    from concourse.bass_utils import run_bass_kernel_spmd

    x = np.asarray(x)
    c = np.asarray(c)
    (W0, b0, Ws0, bs0, Wb0, bb0, W1, b1, Ws1, bs1, Wb1, bb1,
     W2, b2, Ws2, bs2, Wb2, bb2) = (
        np.asarray(a) for a in (W0, b0, Ws0, bs0, Wb0, bb0,
                                W1, b1, Ws1, bs1, Wb1, bb1,
                                W2, b2, Ws2, bs2, Wb2, bb2))
    out_dtype = x.dtype

    if "prog" not in _CACHE:
        _CACHE["prog"] = _build_program()
    nc = _CACHE["prog"]

    # ---- host: FiLM affine params per (weight-set, batch), float64 ----
    sets = [
        _film_params(c, W0, b0, Ws0, bs0, Wb0, bb0),
        _film_params(c, W1, b1, Ws1, bs1, Wb1, bb1),
        _film_params(c, W2, b2, Ws2, bs2, Wb2, bb2),
    ]

    # ---- host: shard + relayout x ----
    # [B, N, 3] -> per core [128, LC] fp16: stream t on partitions
    # 3t..3t+2, ones-row 126, zero-row 127. Stream table per core:
    # 10 full streams per batch + one shared boundary stream per batch
    # pair, switching batch at column BSPLIT.
    xp = np.ascontiguousarray(x, dtype=np.float32)
    xt = np.ascontiguousarray(xp.transpose(0, 2, 1))   # [B, 3, N]

    # (batch_lo, batch_hi, offset): full streams have lo == hi
    stream_table = []
    for pair in range(2):
        ba, bb = 2 * pair, 2 * pair + 1
        stream_table += [(ba, ba, t * LC) for t in range(10)]
        stream_table.append((ba, bb, NFULL))
        stream_table += [(bb, bb, t * LC) for t in range(10)]

    in_maps = []
    for cc in range(NCORES):
        b0 = cc * BPC
        X = np.zeros((P, LC), np.float16)
        for t, (blo, bhi, off) in enumerate(stream_table):
            for c_ in range(D):
                row = 3 * t + c_
                if blo == bhi:
                    X[row] = xt[b0 + blo, c_, off:off + LC]
                else:
                    X[row, :BSPLIT] = xt[b0 + blo, c_, NFULL:N]
                    X[row, BSPLIT:2 * BSPLIT] = xt[b0 + bhi, c_, NFULL:N]
        X[126] = 1.0                # ones-row: carries the bias via matmul
        W6 = np.zeros((P, 6 * P), np.float16)
        for k in range(3):
            A, dv = sets[k]
            for zone in range(2):
                c0 = (k * 2 + zone) * P
                for t, (blo, bhi, off) in enumerate(stream_table):
                    b = b0 + (blo if zone == 0 else bhi)
                    for ci_ in range(3):
                        for cj in range(3):
                            W6[3 * t + cj, c0 + 3 * t + ci_] = \
                                np.float16(A[b, ci_, cj])
                        # bias d rides the ones-row
                        W6[126, c0 + 3 * t + ci_] = np.float16(dv[b, ci_])
                # ones-row regenerates itself: tanh(16.0) == 1.0 in fp16,
                # and the DVE clamp U maps 16.0 -> exactly 1.0 too
                W6[126, c0 + 126] = np.float16(16.0)
        in_maps.append({"x": X, "w": W6})

    res = run_bass_kernel_spmd(nc, in_maps, list(range(NCORES)),
                               trace=bool(PROFILE))
    if PROFILE:
        LAST_EXEC_NS = res.exec_time_ns

    # ---- host: gather + inverse layout ----
    yt = np.empty((B, D, N), np.float32)
    for cc in range(NCORES):
        Y = res.results[cc]["y"]                       # [P, LC] fp16
        b0 = cc * BPC
        for t, (blo, bhi, off) in enumerate(stream_table):
            for c_ in range(D):
                row = 3 * t + c_
                if blo == bhi:
                    yt[b0 + blo, c_, off:off + LC] = Y[row]
                else:
                    yt[b0 + blo, c_, NFULL:N] = Y[row, :BSPLIT]
                    yt[b0 + bhi, c_, NFULL:N] = Y[row, BSPLIT:2 * BSPLIT]
    out = np.ascontiguousarray(yt.transpose(0, 2, 1)).astype(
        out_dtype, copy=False)
    return out


# revision 59
# speedup vs baseline: 1.0274x; 1.0274x over previous
"""Trainium2 Bass kernel for nn_FCond (FiLM-conditioned MLP chain).

Reference computation (B=32, N=100000, D=3, CDIM=128):
    h = x
    for kblk in [0, 1, 2, 2, 2, 2]:
        h = tanh((h @ Wk.T + bk) * sigmoid(c @ Wsk.T + bsk) + (c @ Wbk.T + bbk))

Since the FiLM conditioning depends only on (c, weights), each (batch,
block) reduces to an affine map  h' = tanh(A_kb @ h + d_kb)  with
A_kb [3,3], d_kb [3] precomputed on the host in float64.

Device strategy (pure data parallel over 8 cores, 4 batches/core):
  - Layout: 42 point-streams of 3 comps = 126 partitions, plus a
    constant-1.0 row (126) and a zero row (127). Each batch owns 10
    full streams; each batch PAIR shares one boundary stream that
    switches batch at column BSPLIT (handled by a second weight zone
    and a matmul split at that column).
  - Each block applies a block-diagonal [128x128] fp16 matmul on
    TensorE (PSUM f32), then a nonlinearity evacuating PSUM->SBUF fp16.
  - TWO nonlinearity lanes split the per-block columns (the previous
    all-ACT version was tanh-bound at ~74.4us):
      * ACT lane: 6 chunks x ~1353 cols/block, exact tanh on ScalarE
        (1 col/cyc @ 1.2 GHz + ~310cyc/instr overhead).
      * DVE lane: 2 chunks x 678 cols/block on the otherwise-idle
        VectorE: clamped odd deg-9 minimax polynomial
        y = x*c4(t^2+a1 t+b1)(t^2+a2 t+b2), t = x^2, as 10 DVE
        instrs/chunk using ONLY tensor_scalar (4x mode) and
        tensor_tensor (2x) -- scalar_tensor_tensor has no 2x uop and
        measures 1x. ~4.7 ns/col vs ACT 0.83, so the split is ~14.2%.
        The clamp bound U is the fp16 input where the full fp16 chain
        evaluates to exactly 1.0, so the ones-row (which carries the
        affine bias through the matmul via W[126,col]=d and
        W[126,126]=16) regenerates exactly, like tanh(16)==1.0 does on
        the ACT lane. End-to-end rel err 1.12e-2 (tol 2e-2; the
        deg-11 variant gives 4.9e-3 at ~+1us).
  - PSUM regions are BANK-DISJOINT -- a DVE read of a PSUM bank that
    ACT/PE concurrently touch crashes the exec unit (found by
    bisection; ACT+PE sharing a bank is fine): ACT ping-pong
    [0:1536]/[1536:3072] (banks 0-2/3-5), DVE single slot [3072:...]
    (banks 6-7). Matmul windows split at 512-col bank boundaries and
    BSPLIT. The DVE slot is single-buffered: PE's next V matmul waits
    only for the previous chunk's evac (its first DVE op).
  - Hand-scheduled engine programs: ACT stages rotate in groups
    (0,1)/(2,3)/(4,5) over blocks, DVE stages rotate (V0,V1); PE
    serves both lanes ordered by consumer deadline, with one folded
    act_sem/dve_sem wait per stage covering input-ready + PSUM WAR.
    dve_sem increments only on each DVE stage's evac + final op
    (dense per-op then_inc risks the cayman event-accel deadlock).
  - DMA: weights FIRST (in the all-ACT version the first matmul
    stalled ~3.5us behind x0 on the shared hw queues), then an
    872-col first piece of chunk 0 (split PE/ACT stage 0) so the
    first tanh starts ~11.3us, xV0 early on the scalar queue for the
    V-lane start, bulk transfers + weight set 2 gated out of the
    fill window,
    software-DGE (gpsimd-queued) transfers on their own semaphores
    (their completion order is undefined and CoreSim requires
    sem-start-at-0). Outputs stream per chunk as each last-block
    chunk lands; the final ACT tanh and final DVE chunk are
    column-split so their DMA starts early.

Numerics: weights/bias/activations fp16 (PE @ 1 col/cyc @ 2.4 GHz),
PSUM f32, ACT tanh exact, DVE polynomial max err ~1.2e-2
pre-attenuation. Measured end-to-end rel err 1.12e-2 (tolerance 2e-2).
Measured HW exec: ~67.0-68.2us (all-ACT predecessor 74.4us, original
baseline 78.6us). ACT runs 11.3->60.3us and DVE 13.4->60.3us, both
near-saturated; ~5.5us fixed preamble and ~5.5us output-DMA + barrier
+ NEFF-finalize tail.
"""
import sys
import types

import numpy as np

B, N, D, CDIM = 32, 100000, 3, 128
NCORES = 8
BPC = B // NCORES          # batches per core
P = 128                    # partitions

NST_R = 42                 # streams per core
LC = 9524                  # columns (points per stream)
BSPLIT = 4760              # boundary-stream batch-switch column
NFULL = 95240              # points covered by the 10 full streams

# ---- chunk grid: 6 ACT chunks + 2 DVE chunks per block ----
# PSUM regions must be BANK-DISJOINT: a DVE read of a PSUM bank that
# ACT/PE concurrently touch crashes the exec unit. PSA0 = banks 0-2,
# PSA1 = banks 3-5, PSV = banks 6-7. Even-sized ACT rotation groups fix
# each chunk's psum parity, so even chunks <= 1536 live in PSA0 and odd
# ones in PSA1.
CAS = [1370, 1370, 1368, 1368, 1368, 1368]     # ACT chunk sizes
CV = 656                                       # DVE chunk size
SP0 = 872                  # block-0 chunk-0 split for a fast pipeline fill
NA, NV = 6, 2
A_OFF = [sum(CAS[:i]) for i in range(NA)]
AV = sum(CAS)
V_OFF = [AV + j * CV for j in range(NV)]       # 8008, 8766
PSA_BASE = (0, 1536)       # ACT psum ping-pong bases (banks 0-2 / 3-5)
PSV_BASE = 3072            # DVE psum slot base (banks 6-7)
A_GROUPS = ((0, 1), (2, 3), (4, 5))
WSETS = (0, 1, 2, 2, 2, 2)
TH = (CAS[-1] // 2 + 3) & ~3   # last ACT tanh column split (668)
VH = (CV // 2 + 3) & ~3        # last DVE chunk column split (380)

# ---- deg-9 odd minimax polynomial for tanh on [0,3.25], constrained
# p(3.34)=1 and scaled so the fp16 chain crosses 1.0; factored for
# TS+TT evaluation. U is the fp16 clamp input where the fp16 chain
# yields exactly 1.0 (ones-row trick).
PC4 = 0.00010215534319844812 * 1.0005
PA1 = -26.393814652081367
PB1 = 195.05314722643175
PA2 = -5.005435676115361
PB2 = 48.33620445619426
PU = 3.33203125

PROFILE = False            # set by test harness; collects HW exec time
LAST_EXEC_NS = None

_CACHE = {}


def _install_profile_shim():
    """Register the NTFF profile hook (missing antenv.axon_hooks in this
    container) so run_bass_kernel_spmd(trace=True) can report exec time."""
    if "antenv.axon_hooks" in sys.modules:
        return
    mod = types.ModuleType("antenv.axon_hooks")
    _state = {"hook": None}
    mod.set_axon_ntff_profile_hook = lambda h: _state.__setitem__("hook", h)
    mod.get_axon_ntff_profile_hook = lambda: _state["hook"]
    sys.modules["antenv.axon_hooks"] = mod
    try:
        from trn_agent_boot.trn_boot import _ntff_profile_via_ctypes
        mod.set_axon_ntff_profile_hook(
            _ntff_profile_via_ctypes("/opt/axon/libaxon_pjrt.so"))
    except Exception:
        pass
    import concourse.bass_utils as bu
    bu.upload_artifacts = lambda tmpdir: f"local:{tmpdir}"


def _mm_windows(col0, cols, ps_base):
    """Matmul windows (local lo, local hi, zone): split at PSUM 512-col
    bank boundaries (psum coords) and at the BSPLIT zone switch."""
    cuts = {0, cols}
    b = (ps_base // 512 + 1) * 512
    while b < ps_base + cols:
        cuts.add(b - ps_base)
        b += 512
    if col0 < BSPLIT < col0 + cols:
        cuts.add(BSPLIT - col0)
    cs = sorted(cuts)
    return [(lo, hi, 0 if (col0 + lo) < BSPLIT else 1)
            for lo, hi in zip(cs, cs[1:])]


def _build_program():
    import concourse.bass as bass
    import concourse.bacc as bacc
    from concourse import mybir

    f32 = mybir.dt.float32
    f16 = mybir.dt.float16
    Tanh = mybir.ActivationFunctionType.Tanh
    Alu = mybir.AluOpType

    nc = bacc.Bacc("TRN2", target_bir_lowering=False, debug=False)
    x_d = nc.declare_dram_parameter("x", [P, LC], f16, isOutput=False)
    w_d = nc.declare_dram_parameter("w", [P, 6 * P], f16, isOutput=False)
    y_d = nc.declare_dram_parameter("y", [P, LC], f16, isOutput=True)

    NCH = NA + NV                       # 8 chunks: 0-5 ACT, 6-7 DVE
    C_OFF = A_OFF + V_OFF
    C_LEN = CAS + [CV] * NV

    # static SBUF tensors. ha/hb A-chunk tensors are padded to a
    # 1810-wide partition stride: tanh instructions writing stride-1810
    # dests take a ~181cyc/instr overhead vs ~313cyc at native strides
    # (mechanism unknown, measured repeatedly). These tensors are never
    # DMA'd, so the strided AP can't hit the slow strided-DMA path; the
    # PE pays ~0.9us reading the strided rhs, ACT saves ~4us.
    HPAD = 1810
    def _alloc_hab(nm, c):
        if c < NA:
            return nc.alloc_sbuf_tensor(nm, [P, HPAD],
                                        f16).ap()[:, 0:C_LEN[c]]
        return nc.alloc_sbuf_tensor(nm, [P, C_LEN[c]], f16).ap()
    xin = [nc.alloc_sbuf_tensor(f"xin{c}", [P, C_LEN[c]], f16).ap()
           for c in range(NCH)]
    ha = [_alloc_hab(f"ha{c}", c) for c in range(NCH)]
    hb = [_alloc_hab(f"hb{c}", c) for c in range(NCH)]
    yout = [nc.alloc_sbuf_tensor(f"yout{c}", [P, C_LEN[c]], f16).ap()
            for c in range(NCH)]
    wall = nc.alloc_sbuf_tensor("wall", [P, 6 * P], f16).ap()
    ps = nc.alloc_psum_tensor("ps", [P, 4096], f32).ap()
    # DVE polynomial temporaries
    vt = {n: nc.alloc_sbuf_tensor(f"vt_{n}", [P, CV], f16).ap()
          for n in ("xe", "t", "w", "u1", "u2", "f", "p", "p2")}

    # ---- stage tables ----
    a_stages = [(k, ci) for grp in A_GROUPS for k in range(6) for ci in grp]
    v_stages = [(k, j) for k in range(6) for j in range(NV)]
    idx_a = {kc: s for s, kc in enumerate(a_stages)}
    idx_v = {kc: s for s, kc in enumerate(v_stages)}
    NSA, NSV = len(a_stages), len(v_stages)

    # ACT instruction counts (stage 0 and the last stage are split)
    a_n = [2 if s in (0, NSA - 1) else 1 for s in range(NSA)]
    a_cum = np.cumsum(a_n).tolist()          # act_sem value after stage s
    # dve_sem incs per stage: evac + final y only (sparse then_inc: dense
    # per-op incs can trip the cayman event-accel deadlock); last stage's
    # final mult is split in two, each inc'ing
    v_n = [3 if s == NSV - 1 else 2 for s in range(NSV)]
    v_cum = np.cumsum(v_n).tolist()          # dve_sem value after stage sv

    # PE order: merge lanes by estimated consumer deadline
    EST_A, EST_V = 1408.0, 4107.0
    # A-stage 0 is split into two PE stages (Aa/Ab) for the fill
    ev = ([("Aa", 0, 0.0), ("Ab", 0, 730.0)]
          + [("A", s, s * EST_A) for s in range(1, NSA)]
          + [("V", sv, sv * EST_V + 500.0) for sv in range(NSV)])
    pe_order = sorted(ev, key=lambda e: e[2])
    pe_pos = {(lane, i): p for p, (lane, i, _) in enumerate(pe_order)}

    def h_in(k, c):
        if k == 0:
            return xin[c]
        return ha[c] if k % 2 == 1 else hb[c]

    def h_out(k, c):
        if k == 5:
            return yout[c]
        return ha[c] if k % 2 == 0 else hb[c]

    H = P // 2

    with (
        nc.Block(no_gpsimd_drain=True) as block,
        nc.semaphore("dxa0") as dxa0,
        nc.semaphore("dxa1") as dxa1,
        nc.semaphore("dxa2") as dxa2,
        nc.semaphore("dxa3") as dxa3,
        nc.semaphore("dxa4") as dxa4,
        nc.semaphore("dxa5") as dxa5,
        nc.semaphore("dxa0a") as dxa0a,
        nc.semaphore("dxv0") as dxv0,
        nc.semaphore("dxv1") as dxv1,
        nc.semaphore("dww") as dww,
        nc.semaphore("dww12") as dww12,
        nc.semaphore("dout") as dout,
        nc.semaphore("dxg") as dxg,
        nc.semaphore("doutg") as doutg,
        nc.semaphore("act_sem") as act_sem,
        nc.semaphore("pe_sem") as pe_sem,
        nc.semaphore("dve_sem") as dve_sem,
    ):
        dxa = [dxa0, dxa1, dxa2, dxa3, dxa4, dxa5]
        dxv = [dxv0, dxv1]

        def x_dma(eng, c, p0, p1, sem):
            eng.dma_start(out=xin[c][p0:p1, :],
                          in_=x_d[p0:p1, C_OFF[c]:C_OFF[c] + C_LEN[c]]
                          ).then_inc(sem, 16)

        def y_dma(eng, c, sem, req, p0=0, p1=P, c0=0, c1=None, dsem=None):
            c1 = C_LEN[c] if c1 is None else c1
            eng.dma_start(out=y_d[p0:p1, C_OFF[c] + c0:C_OFF[c] + c1],
                          in_=yout[c][p0:p1, c0:c1]
                          )._wait_ge(sem, req).then_inc(dsem or dout, 16)

        N_OUT = 6   # output transfers on the sync/scalar queues
        N_OUTG = 7  # output transfers on the gpsimd (software-DGE) queue

        @block.sync
        def _(sync: bass.BassEngine):
            # weights FIRST: the first matmul needs w; in the baseline w
            # queued behind the big x0 transfer on the shared hw queues
            # and the fill cost ~9us.
            sync.dma_start(out=wall[0:H, 0:2 * P], in_=w_d[0:H, 0:2 * P]
                           ).then_inc(dww, 16)
            # small first piece of chunk 0: the first matmul+tanh only
            # need [0:512], which lands ~1.5us before the full chunk
            sync.dma_start(out=xin[0][0:H, 0:SP0], in_=x_d[0:H, 0:SP0]
                           ).then_inc(dxa0a, 16)
            sync.dma_start(out=xin[0][0:H, SP0:C_LEN[0]],
                           in_=x_d[0:H, SP0:C_LEN[0]]).then_inc(dxa0, 16)
            x_dma(sync, 1, 0, H, dxa1)
            # weight set 1 now; set 2 (first needed by block 2, ~19us)
            # moves out of the fill window. Both ride the sync queue so
            # their completion order matches issue order.
            sync.dma_start(out=wall[:, 2 * P:4 * P],
                           in_=w_d[:, 2 * P:4 * P]).then_inc(dww12, 16)
            # keep bulk transfers out of the fill window
            sync.wait_ge(dxa1, 32)
            sync.dma_start(out=wall[:, 4 * P:6 * P],
                           in_=w_d[:, 4 * P:6 * P]).then_inc(dww12, 16)
            x_dma(sync, NA + 1, 0, P, dxv1)     # xV1 whole
            x_dma(sync, 2, 0, H, dxa2)
            x_dma(sync, 3, 0, H, dxa3)
            x_dma(sync, 4, 0, H, dxa4)
            x_dma(sync, 5, 0, H, dxa5)
            # output tail (in readiness order; queue is in-order)
            y_dma(sync, 3, act_sem, a_cum[idx_a[(5, 3)]])
            y_dma(sync, 4, act_sem, a_cum[idx_a[(5, 4)]], 0, H)
            y_dma(sync, 5, act_sem, a_cum[NSA - 1] - 1, 0, H, 0, TH)
            y_dma(sync, 5, act_sem, a_cum[NSA - 1], 0, H, TH, CAS[-1])
            sync.wait_ge(dout, 16 * N_OUT)
            sync.wait_ge(doutg, 16 * N_OUTG)

        @block.scalar
        def _(scalar: bass.BassEngine):
            scalar.dma_start(out=wall[H:P, 0:2 * P], in_=w_d[H:P, 0:2 * P]
                             ).then_inc(dww, 16)
            scalar.dma_start(out=xin[0][H:P, 0:SP0], in_=x_d[H:P, 0:SP0]
                             ).then_inc(dxa0a, 16)
            scalar.dma_start(out=xin[0][H:P, SP0:C_LEN[0]],
                             in_=x_d[H:P, SP0:C_LEN[0]]).then_inc(dxa0, 16)
            x_dma(scalar, NA, 0, P, dxv0)       # xV0 whole
            x_dma(scalar, 1, H, P, dxa1)
            for s, (k, ci) in enumerate(a_stages):
                base = PSA_BASE[s % 2]
                cl = C_LEN[ci]
                if s == 0:
                    for (a, b_), key in (((0, SP0), "Aa"), ((SP0, cl),
                                                           "Ab")):
                        scalar.activation(
                            h_out(k, ci)[:, a:b_], ps[:, base + a:base + b_],
                            Tanh, bias=0.0, scale=1.0,
                        )._wait_ge(pe_sem, pe_pos[(key, 0)] + 1
                                   ).then_inc(act_sem, 1)
                elif s == NSA - 1:
                    for a, b_ in ((0, TH), (TH, cl)):
                        act = scalar.activation(
                            h_out(k, ci)[:, a:b_], ps[:, base + a:base + b_],
                            Tanh, bias=0.0, scale=1.0)
                        if a == 0:
                            act._wait_ge(pe_sem, pe_pos[("A", s)] + 1)
                        act.then_inc(act_sem, 1)
                else:
                    scalar.activation(
                        h_out(k, ci), ps[:, base:base + cl], Tanh,
                        bias=0.0, scale=1.0,
                    )._wait_ge(pe_sem, pe_pos[("A", s)] + 1
                               ).then_inc(act_sem, 1)
            # remaining output quarters on the now-idle ACT queue
            y_dma(scalar, 5, act_sem, a_cum[NSA - 1], H, P, TH, CAS[-1])
            y_dma(scalar, NA + 1, dve_sem, v_cum[NSV - 1], 0, P, VH, CV)

        @block.gpsimd
        def _(g: bass.BassEngine):
            # stay clear of the fill window; software-DGE transfers use
            # their own semaphores (dxg cumulative in queue order)
            g.wait_ge(dxa1, 32)
            x_dma(g, 2, H, P, dxg)
            x_dma(g, 3, H, P, dxg)
            x_dma(g, 4, H, P, dxg)
            x_dma(g, 5, H, P, dxg)
            y_dma(g, 0, act_sem, a_cum[idx_a[(5, 0)]], dsem=doutg)
            y_dma(g, 1, act_sem, a_cum[idx_a[(5, 1)]], dsem=doutg)
            y_dma(g, 2, act_sem, a_cum[idx_a[(5, 2)]], dsem=doutg)
            y_dma(g, NA, dve_sem, v_cum[idx_v[(5, 0)]], dsem=doutg)
            y_dma(g, 4, act_sem, a_cum[idx_a[(5, 4)]], H, P, dsem=doutg)
            y_dma(g, 5, act_sem, a_cum[NSA - 1] - 1, H, P, 0, TH,
                  dsem=doutg)
            y_dma(g, NA + 1, dve_sem, v_cum[NSV - 1] - 1, 0, P, 0, VH,
                  dsem=doutg)

        def _vector_body(v: bass.BassEngine):
            psv = ps[:, PSV_BASE:PSV_BASE + CV]
            xe, t, w, u1, u2, f, p, p2 = (vt[n] for n in
                                          ("xe", "t", "w", "u1", "u2",
                                           "f", "p", "p2"))
            for sv, (k, j) in enumerate(v_stages):
                v.wait_ge(pe_sem, pe_pos[("V", sv)] + 1)
                # evac + clamp from PSUM f32 -> SBUF fp16; within-engine
                # deps below ride program order (the DVE pipe drains
                # between ops), so only the evac and the final y inc
                # dve_sem for PE / DMA consumers
                v.tensor_scalar(xe, psv, PU, -PU, Alu.min, Alu.max
                                ).then_inc(dve_sem, 1)
                # factored deg-9: all TT (2x) / TS (4x); STT would
                # run at 1x (no 2x uop) so each (a+s)*b is TS+TT
                v.tensor_tensor(t, xe, xe, Alu.mult)
                v.tensor_scalar(w, t, PA1, None, Alu.add)
                v.tensor_tensor(u1, w, t, Alu.mult)
                v.tensor_scalar(w, t, PA2, None, Alu.add)
                v.tensor_tensor(u2, w, t, Alu.mult)
                v.tensor_scalar(f, u1, PB1, PC4, Alu.add, Alu.mult)
                v.tensor_scalar(w, u2, PB2, None, Alu.add)
                v.tensor_tensor(p2, w, f, Alu.mult)
                out = h_out(k, NA + j)
                if sv == NSV - 1:
                    v.tensor_tensor(out[:, 0:VH], p2[:, 0:VH], xe[:, 0:VH],
                                    Alu.mult).then_inc(dve_sem, 1)
                    v.tensor_tensor(out[:, VH:CV], p2[:, VH:CV],
                                    xe[:, VH:CV], Alu.mult
                                    ).then_inc(dve_sem, 1)
                else:
                    v.tensor_tensor(out, p2, xe, Alu.mult
                                    ).then_inc(dve_sem, 1)

        block.vector(_vector_body)

        @block.tensor
        def _(tensor: bass.BassEngine):
            tensor.wait_ge(dww, 32)
            seen1 = seen2 = False
            for lane, i, _t in pe_order:
                if lane in ("A", "Aa", "Ab"):
                    s = i
                    k, ci = a_stages[s]
                    base = PSA_BASE[s % 2]
                    coff, clen = A_OFF[ci], C_LEN[ci]
                    if lane == "Aa":
                        clen = SP0
                    elif lane == "Ab":
                        base, coff = base + SP0, coff + SP0
                        clen = C_LEN[ci] - SP0
                    rhs = h_in(k, ci)
                    if lane == "Ab":
                        rhs = rhs[:, SP0:]
                else:
                    sv = i
                    k, j = v_stages[sv]
                    base = PSV_BASE
                    coff, clen = V_OFF[j], CV
                    rhs = h_in(k, NA + j)
                ks = WSETS[k]
                if ks >= 1 and not seen1:
                    tensor.wait_ge(dww12, 16)   # set 1
                    seen1 = True
                if ks == 2 and not seen2:
                    tensor.wait_ge(dww12, 32)   # + set 2 (same queue)
                    seen2 = True
                if k == 0:
                    if lane != "V":
                        if lane == "Aa":
                            tensor.wait_ge(dxa0a, 32)
                        elif lane == "Ab":
                            tensor.wait_ge(dxa[0], 32)
                        elif ci < 2 and s > 0:
                            tensor.wait_ge(dxa[ci], 32)
                        elif ci >= 2:
                            # top half on sync; bottom halves ride the
                            # gpsimd software-DGE queue whose completion
                            # order is not guaranteed -> wait for all 4
                            # (they land ~16us; first needed ~26us)
                            tensor.wait_ge(dxa[ci], 16)
                            tensor.wait_ge(dxg, 64)
                    else:
                        tensor.wait_ge(dxv[j], 16)
                # folded input-ready + PSUM WAR wait
                wv = 0
                if lane in ("A", "Aa", "Ab"):
                    if s >= 2:
                        wv = a_cum[s - 2]
                    if k > 0:
                        wv = max(wv, a_cum[idx_a[(k - 1, ci)]])
                else:
                    if sv >= 1:
                        wv = (v_cum[sv - 2] + 1) if sv >= 2 else 1
                wins = _mm_windows(coff, clen, base)
                for wi, (lo, hi, zone) in enumerate(wins):
                    kz = ks * 2 + zone
                    mm = tensor.matmul(ps[:, base + lo:base + hi],
                                       wall[:, kz * P:(kz + 1) * P],
                                       rhs[:, lo:hi],
                                       start=True, stop=True)
                    if wi == 0 and wv > 0:
                        if lane == "V":
                            mm._wait_ge(dve_sem, wv)
                        else:
                            mm._wait_ge(act_sem, wv)
                mm.then_inc(pe_sem, 1)

    nc.compile()
    return nc


def _film_params(c, Wk, bk, Wsk, bsk, Wbk, bbk):
    """A[b] = diag(scale[b]) @ Wk ; d[b] = scale[b]*bk + shift[b], float64."""
    c = c.astype(np.float64)
    scale = 1.0 / (1.0 + np.exp(-(c @ Wsk.astype(np.float64).T
                                  + bsk.astype(np.float64))))     # [B,3]
    shift = c @ Wbk.astype(np.float64).T + bbk.astype(np.float64)  # [B,3]
    A = scale[:, :, None] * Wk.astype(np.float64)[None]            # [B,3,3]
    d = scale * bk.astype(np.float64) + shift                      # [B,3]
    return A, d


def kernel(t, x, c,
           W0, b0, Ws0, bs0, Wb0, bb0,
           W1, b1, Ws1, bs1, Wb1, bb1,
           W2, b2, Ws2, bs2, Wb2, bb2):
    global LAST_EXEC_NS
    if PROFILE:
        _install_profile_shim()
    from concourse.bass_utils import run_bass_kernel_spmd

    x = np.asarray(x)
    c = np.asarray(c)
    (W0, b0, Ws0, bs0, Wb0, bb0, W1, b1, Ws1, bs1, Wb1, bb1,
     W2, b2, Ws2, bs2, Wb2, bb2) = (
        np.asarray(a) for a in (W0, b0, Ws0, bs0, Wb0, bb0,
                                W1, b1, Ws1, bs1, Wb1, bb1,
                                W2, b2, Ws2, bs2, Wb2, bb2))
    out_dtype = x.dtype

    if "prog" not in _CACHE:
        _CACHE["prog"] = _build_program()
    nc = _CACHE["prog"]

    # ---- host: FiLM affine params per (weight-set, batch), float64 ----
    sets = [
        _film_params(c, W0, b0, Ws0, bs0, Wb0, bb0),
        _film_params(c, W1, b1, Ws1, bs1, Wb1, bb1),
        _film_params(c, W2, b2, Ws2, bs2, Wb2, bb2),
    ]

    # ---- host: shard + relayout x ----
    # [B, N, 3] -> per core [128, LC] fp16: stream t on partitions
    # 3t..3t+2, ones-row 126, zero-row 127. Stream table per core:
    # 10 full streams per batch + one shared boundary stream per batch
    # pair, switching batch at column BSPLIT.
    xp = np.ascontiguousarray(x, dtype=np.float32)
    xt = np.ascontiguousarray(xp.transpose(0, 2, 1))   # [B, 3, N]

    # (batch_lo, batch_hi, offset): full streams have lo == hi
    stream_table = []
    for pair in range(2):
        ba, bb = 2 * pair, 2 * pair + 1
        stream_table += [(ba, ba, t * LC) for t in range(10)]
        stream_table.append((ba, bb, NFULL))
        stream_table += [(bb, bb, t * LC) for t in range(10)]

    in_maps = []
    for cc in range(NCORES):
        b0 = cc * BPC
        X = np.zeros((P, LC), np.float16)
        for t, (blo, bhi, off) in enumerate(stream_table):
            for c_ in range(D):
                row = 3 * t + c_
                if blo == bhi:
                    X[row] = xt[b0 + blo, c_, off:off + LC]
                else:
                    X[row, :BSPLIT] = xt[b0 + blo, c_, NFULL:N]
                    X[row, BSPLIT:2 * BSPLIT] = xt[b0 + bhi, c_, NFULL:N]
        X[126] = 1.0                # ones-row: carries the bias via matmul
        W6 = np.zeros((P, 6 * P), np.float16)
        for k in range(3):
            A, dv = sets[k]
            for zone in range(2):
                c0 = (k * 2 + zone) * P
                for t, (blo, bhi, off) in enumerate(stream_table):
                    b = b0 + (blo if zone == 0 else bhi)
                    for ci_ in range(3):
                        for cj in range(3):
                            W6[3 * t + cj, c0 + 3 * t + ci_] = \
                                np.float16(A[b, ci_, cj])
                        # bias d rides the ones-row
                        W6[126, c0 + 3 * t + ci_] = np.float16(dv[b, ci_])
                # ones-row regenerates itself: tanh(16.0) == 1.0 in fp16,
                # and the DVE clamp U maps 16.0 -> exactly 1.0 too
                W6[126, c0 + 126] = np.float16(16.0)
        in_maps.append({"x": X, "w": W6})

    res = run_bass_kernel_spmd(nc, in_maps, list(range(NCORES)),
                               trace=bool(PROFILE))
    if PROFILE:
        LAST_EXEC_NS = res.exec_time_ns

    # ---- host: gather + inverse layout ----
    yt = np.empty((B, D, N), np.float32)
    for cc in range(NCORES):
        Y = res.results[cc]["y"]                       # [P, LC] fp16
        b0 = cc * BPC
        for t, (blo, bhi, off) in enumerate(stream_table):
            for c_ in range(D):
                row = 3 * t + c_
                if blo == bhi:
                    yt[b0 + blo, c_, off:off + LC] = Y[row]
                else:
                    yt[b0 + blo, c_, NFULL:N] = Y[row, :BSPLIT]
                    yt[b0 + bhi, c_, NFULL:N] = Y[row, BSPLIT:2 * BSPLIT]
    out = np.ascontiguousarray(yt.transpose(0, 2, 1)).astype(
        out_dtype, copy=False)
    return out


# revision 62
# speedup vs baseline: 1.0425x; 1.0147x over previous
"""Trainium2 Bass kernel for nn_FCond (FiLM-conditioned MLP chain).

Reference computation (B=32, N=100000, D=3, CDIM=128):
    h = x
    for kblk in [0, 1, 2, 2, 2, 2]:
        h = tanh((h @ Wk.T + bk) * sigmoid(c @ Wsk.T + bsk) + (c @ Wbk.T + bbk))

Since the FiLM conditioning depends only on (c, weights), each (batch,
block) reduces to an affine map  h' = tanh(A_kb @ h + d_kb)  with
A_kb [3,3], d_kb [3] precomputed on the host in float64.

Device strategy (pure data parallel over 8 cores, 4 batches/core):
  - Layout: 42 point-streams of 3 comps = 126 partitions, plus a
    constant-1.0 row (126) and a zero row (127). Each batch owns 10
    full streams; each batch PAIR shares one boundary stream that
    switches batch at column BSPLIT (handled by a second weight zone
    and a matmul split at that column).
  - Each block applies a block-diagonal [128x128] fp16 matmul on
    TensorE (PSUM f32), then a nonlinearity evacuating PSUM->SBUF fp16.
  - TWO nonlinearity lanes split the per-block columns (the previous
    all-ACT version was tanh-bound at ~74.4us):
      * ACT lane: 6 chunks x ~1369 cols/block, exact tanh on ScalarE
        (1 col/cyc @ 1.2 GHz; ~181cyc/instr overhead via the
        stride-1810 dest fast path, vs ~313cyc at native strides).
      * DVE lane: 2 chunks x 656 cols/block on the otherwise-idle
        VectorE: clamped odd deg-9 minimax polynomial
        y = x*c4(t^2+a1 t+b1)(t^2+a2 t+b2), t = x^2, as 10 DVE
        instrs/chunk using ONLY tensor_scalar (4x mode) and
        tensor_tensor (2x) -- scalar_tensor_tensor has no 2x uop and
        measures 1x. ~4.7 ns/col vs ACT 0.83, so the split is ~14.2%.
        The clamp bound U is the fp16 input where the full fp16 chain
        evaluates to exactly 1.0, so the ones-row (which carries the
        affine bias through the matmul via W[126,col]=d and
        W[126,126]=16) regenerates exactly, like tanh(16)==1.0 does on
        the ACT lane. End-to-end rel err 1.12e-2 (tol 2e-2; the
        deg-11 variant gives 4.9e-3 at ~+1us).
  - PSUM regions are BANK-DISJOINT -- a DVE read of a PSUM bank that
    ACT/PE concurrently touch crashes the exec unit (found by
    bisection; ACT+PE sharing a bank is fine): ACT ping-pong
    [0:1536]/[1536:3072] (banks 0-2/3-5), DVE single slot [3072:...]
    (banks 6-7). Matmul windows split at 512-col bank boundaries and
    BSPLIT. The DVE slot is single-buffered: PE's next V matmul waits
    only for the previous chunk's evac (its first DVE op).
  - Hand-scheduled engine programs: ACT stages rotate in groups
    (0,1)/(2,3)/(4,5) over blocks, DVE stages rotate (V0,V1); PE
    serves both lanes ordered by consumer deadline, with one folded
    act_sem/dve_sem wait per stage covering input-ready + PSUM WAR.
    dve_sem increments only on each DVE stage's evac + final op
    (dense per-op then_inc risks the cayman event-accel deadlock).
  - DMA: weights FIRST (in the all-ACT version the first matmul
    stalled ~3.5us behind x0 on the shared hw queues), then an
    872-col first piece of chunk 0 (split PE/ACT stage 0) so the
    first tanh starts ~11.3us, xV0 early on the scalar queue for the
    V-lane start, bulk transfers + weight set 2 gated out of the
    fill window,
    software-DGE (gpsimd-queued) transfers on their own semaphores
    (their completion order is undefined and CoreSim requires
    sem-start-at-0). Outputs stream per chunk as each last-block
    chunk lands; the final ACT tanh and final DVE chunk are
    column-split so their DMA starts early.

Numerics: weights/bias/activations fp16 (PE @ 1 col/cyc @ 2.4 GHz),
PSUM f32, ACT tanh exact, DVE polynomial max err ~1.2e-2
pre-attenuation. Measured end-to-end rel err 1.12e-2 (tolerance 2e-2).
Measured HW exec: ~67.0-68.2us (all-ACT predecessor 74.4us, original
baseline 78.6us). ACT runs 11.3->60.3us and DVE 13.4->60.3us, both
near-saturated; ~5.5us fixed preamble and ~5.5us output-DMA + barrier
+ NEFF-finalize tail.
"""
import sys
import types

import numpy as np

B, N, D, CDIM = 32, 100000, 3, 128
NCORES = 8
BPC = B // NCORES          # batches per core
P = 128                    # partitions

NST_R = 42                 # streams per core
LC = 9524                  # columns (points per stream)
BSPLIT = 4760              # boundary-stream batch-switch column
NFULL = 95240              # points covered by the 10 full streams

# ---- chunk grid: 6 ACT chunks + 2 DVE chunks per block ----
# PSUM regions must be BANK-DISJOINT: a DVE read of a PSUM bank that
# ACT/PE concurrently touch crashes the exec unit. PSA0 = banks 0-2,
# PSA1 = banks 3-5, PSV = banks 6-7. Even-sized ACT rotation groups fix
# each chunk's psum parity, so even chunks <= 1536 live in PSA0 and odd
# ones in PSA1.
CAS = [1370, 1370, 1368, 1368, 1368, 1368]     # ACT chunk sizes
CV = 656                                       # DVE chunk size
SP0 = 872                  # block-0 chunk-0 split for a fast pipeline fill
NA, NV = 6, 2
A_OFF = [sum(CAS[:i]) for i in range(NA)]
AV = sum(CAS)
V_OFF = [AV + j * CV for j in range(NV)]       # 8008, 8766
PSA_BASE = (0, 1536)       # ACT psum ping-pong bases (banks 0-2 / 3-5)
PSV_BASE = 3072            # DVE psum slot base (banks 6-7)
A_GROUPS = ((0, 1), (2, 3), (4, 5))
WSETS = (0, 1, 2, 2, 2, 2)
TH = (CAS[-1] // 2 + 3) & ~3   # last ACT tanh column split (668)
VH = (CV // 2 + 3) & ~3        # last DVE chunk column split (380)

# ---- deg-9 odd minimax polynomial for tanh on [0,3.25], constrained
# p(3.34)=1 and scaled so the fp16 chain crosses 1.0; factored for
# TS+TT evaluation. U is the fp16 clamp input where the fp16 chain
# yields exactly 1.0 (ones-row trick).
PC4 = 0.00010215534319844812 * 1.0005
PA1 = -26.393814652081367
PB1 = 195.05314722643175
PA2 = -5.005435676115361
PB2 = 48.33620445619426
PU = 3.33203125

PROFILE = False            # set by test harness; collects HW exec time
LAST_EXEC_NS = None

_CACHE = {}


def _install_profile_shim():
    """Register the NTFF profile hook (missing antenv.axon_hooks in this
    container) so run_bass_kernel_spmd(trace=True) can report exec time."""
    if "antenv.axon_hooks" in sys.modules:
        return
    mod = types.ModuleType("antenv.axon_hooks")
    _state = {"hook": None}
    mod.set_axon_ntff_profile_hook = lambda h: _state.__setitem__("hook", h)
    mod.get_axon_ntff_profile_hook = lambda: _state["hook"]
    sys.modules["antenv.axon_hooks"] = mod
    try:
        from trn_agent_boot.trn_boot import _ntff_profile_via_ctypes
        mod.set_axon_ntff_profile_hook(
            _ntff_profile_via_ctypes("/opt/axon/libaxon_pjrt.so"))
    except Exception:
        pass
    import concourse.bass_utils as bu
    bu.upload_artifacts = lambda tmpdir: f"local:{tmpdir}"


def _mm_windows(col0, cols, ps_base):
    """Matmul windows (local lo, local hi, zone): split at PSUM 512-col
    bank boundaries (psum coords) and at the BSPLIT zone switch."""
    cuts = {0, cols}
    b = (ps_base // 512 + 1) * 512
    while b < ps_base + cols:
        cuts.add(b - ps_base)
        b += 512
    if col0 < BSPLIT < col0 + cols:
        cuts.add(BSPLIT - col0)
    cs = sorted(cuts)
    return [(lo, hi, 0 if (col0 + lo) < BSPLIT else 1)
            for lo, hi in zip(cs, cs[1:])]


def _build_program():
    import concourse.bass as bass
    import concourse.bacc as bacc
    from concourse import mybir

    f32 = mybir.dt.float32
    f16 = mybir.dt.float16
    Tanh = mybir.ActivationFunctionType.Tanh
    Alu = mybir.AluOpType

    nc = bacc.Bacc("TRN2", target_bir_lowering=False, debug=False)
    x_d = nc.declare_dram_parameter("x", [P, LC], f16, isOutput=False)
    w_d = nc.declare_dram_parameter("w", [P, 6 * P], f16, isOutput=False)
    y_d = nc.declare_dram_parameter("y", [P, LC], f16, isOutput=True)

    NCH = NA + NV                       # 8 chunks: 0-5 ACT, 6-7 DVE
    C_OFF = A_OFF + V_OFF
    C_LEN = CAS + [CV] * NV

    # static SBUF tensors. ha/hb A-chunk tensors are padded to a
    # 1810-wide partition stride: tanh instructions writing stride-1810
    # dests take a ~181cyc/instr overhead vs ~313cyc at native strides
    # (mechanism unknown, measured repeatedly). These tensors are never
    # DMA'd, so the strided AP can't hit the slow strided-DMA path; the
    # PE pays ~0.9us reading the strided rhs, ACT saves ~4us.
    HPAD = 1810
    def _alloc_hab(nm, c):
        if c < NA:
            return nc.alloc_sbuf_tensor(nm, [P, HPAD],
                                        f16).ap()[:, 0:C_LEN[c]]
        return nc.alloc_sbuf_tensor(nm, [P, C_LEN[c]], f16).ap()
    xin = [nc.alloc_sbuf_tensor(f"xin{c}", [P, C_LEN[c]], f16).ap()
           for c in range(NCH)]
    ha = [_alloc_hab(f"ha{c}", c) for c in range(NCH)]
    hb = [_alloc_hab(f"hb{c}", c) for c in range(NCH)]
    yout = [nc.alloc_sbuf_tensor(f"yout{c}", [P, C_LEN[c]], f16).ap()
            for c in range(NCH)]
    wall = nc.alloc_sbuf_tensor("wall", [P, 6 * P], f16).ap()
    ps = nc.alloc_psum_tensor("ps", [P, 4096], f32).ap()
    # DVE polynomial temporaries
    vt = {n: nc.alloc_sbuf_tensor(f"vt_{n}", [P, CV], f16).ap()
          for n in ("xe", "t", "w", "u1", "u2", "f", "p", "p2")}

    # ---- stage tables ----
    a_stages = [(k, ci) for grp in A_GROUPS for k in range(6) for ci in grp]
    v_stages = [(k, j) for k in range(6) for j in range(NV)]
    idx_a = {kc: s for s, kc in enumerate(a_stages)}
    idx_v = {kc: s for s, kc in enumerate(v_stages)}
    NSA, NSV = len(a_stages), len(v_stages)

    # ACT instruction counts (stage 0 and the last stage are split)
    a_n = [2 if s in (0, NSA - 1) else 1 for s in range(NSA)]
    a_cum = np.cumsum(a_n).tolist()          # act_sem value after stage s
    # dve_sem incs per stage: evac + final y only (sparse then_inc: dense
    # per-op incs can trip the cayman event-accel deadlock); last stage's
    # final mult is split in two, each inc'ing
    v_n = [3 if s == NSV - 1 else 2 for s in range(NSV)]
    v_cum = np.cumsum(v_n).tolist()          # dve_sem value after stage sv

    # PE order: merge lanes by estimated consumer deadline
    EST_A, EST_V = 1408.0, 4107.0
    # A-stage 0 is split into two PE stages (Aa/Ab) for the fill
    ev = ([("Aa", 0, 0.0), ("Ab", 0, 730.0)]
          + [("A", s, s * EST_A) for s in range(1, NSA)]
          + [("V", sv, sv * EST_V + 500.0) for sv in range(NSV)])
    pe_order = sorted(ev, key=lambda e: e[2])
    pe_pos = {(lane, i): p for p, (lane, i, _) in enumerate(pe_order)}

    def h_in(k, c):
        if k == 0:
            return xin[c]
        return ha[c] if k % 2 == 1 else hb[c]

    def h_out(k, c):
        if k == 5:
            return yout[c]
        return ha[c] if k % 2 == 0 else hb[c]

    H = P // 2

    with (
        nc.Block(no_gpsimd_drain=True) as block,
        nc.semaphore("dxa0") as dxa0,
        nc.semaphore("dxa1") as dxa1,
        nc.semaphore("dxa2") as dxa2,
        nc.semaphore("dxa3") as dxa3,
        nc.semaphore("dxa4") as dxa4,
        nc.semaphore("dxa5") as dxa5,
        nc.semaphore("dxa0a") as dxa0a,
        nc.semaphore("dxv0") as dxv0,
        nc.semaphore("dxv1") as dxv1,
        nc.semaphore("dww") as dww,
        nc.semaphore("dww12") as dww12,
        nc.semaphore("dout") as dout,
        nc.semaphore("dxg") as dxg,
        nc.semaphore("doutg") as doutg,
        nc.semaphore("act_sem") as act_sem,
        nc.semaphore("pe_sem") as pe_sem,
        nc.semaphore("dve_sem") as dve_sem,
    ):
        dxa = [dxa0, dxa1, dxa2, dxa3, dxa4, dxa5]
        dxv = [dxv0, dxv1]

        def x_dma(eng, c, p0, p1, sem):
            eng.dma_start(out=xin[c][p0:p1, :],
                          in_=x_d[p0:p1, C_OFF[c]:C_OFF[c] + C_LEN[c]]
                          ).then_inc(sem, 16)

        def y_dma(eng, c, sem, req, p0=0, p1=P, c0=0, c1=None, dsem=None):
            c1 = C_LEN[c] if c1 is None else c1
            eng.dma_start(out=y_d[p0:p1, C_OFF[c] + c0:C_OFF[c] + c1],
                          in_=yout[c][p0:p1, c0:c1]
                          )._wait_ge(sem, req).then_inc(dsem or dout, 16)

        N_OUT = 6   # output transfers on the sync/scalar queues
        N_OUTG = 7  # output transfers on the gpsimd (software-DGE) queue

        @block.sync
        def _(sync: bass.BassEngine):
            # weights FIRST: the first matmul needs w; in the baseline w
            # queued behind the big x0 transfer on the shared hw queues
            # and the fill cost ~9us.
            sync.dma_start(out=wall[0:H, 0:2 * P], in_=w_d[0:H, 0:2 * P]
                           ).then_inc(dww, 16)
            # small first piece of chunk 0: the first matmul+tanh only
            # need [0:512], which lands ~1.5us before the full chunk
            sync.dma_start(out=xin[0][0:H, 0:SP0], in_=x_d[0:H, 0:SP0]
                           ).then_inc(dxa0a, 16)
            sync.dma_start(out=xin[0][0:H, SP0:C_LEN[0]],
                           in_=x_d[0:H, SP0:C_LEN[0]]).then_inc(dxa0, 16)
            x_dma(sync, 1, 0, H, dxa1)
            # weight set 1 now; set 2 (first needed by block 2, ~19us)
            # moves out of the fill window. Both ride the sync queue so
            # their completion order matches issue order.
            sync.dma_start(out=wall[:, 2 * P:4 * P],
                           in_=w_d[:, 2 * P:4 * P]).then_inc(dww12, 16)
            # keep bulk transfers out of the fill window
            sync.wait_ge(dxa1, 32)
            sync.dma_start(out=wall[:, 4 * P:6 * P],
                           in_=w_d[:, 4 * P:6 * P]).then_inc(dww12, 16)
            x_dma(sync, NA + 1, 0, P, dxv1)     # xV1 whole
            x_dma(sync, 2, 0, H, dxa2)
            x_dma(sync, 3, 0, H, dxa3)
            x_dma(sync, 4, 0, H, dxa4)
            x_dma(sync, 5, 0, H, dxa5)
            # output tail (in readiness order; queue is in-order)
            y_dma(sync, 3, act_sem, a_cum[idx_a[(5, 3)]])
            y_dma(sync, 4, act_sem, a_cum[idx_a[(5, 4)]], 0, H)
            y_dma(sync, 5, act_sem, a_cum[NSA - 1] - 1, 0, H, 0, TH)
            y_dma(sync, 5, act_sem, a_cum[NSA - 1], 0, H, TH, CAS[-1])
            sync.wait_ge(dout, 16 * N_OUT)
            sync.wait_ge(doutg, 16 * N_OUTG)

        @block.scalar
        def _(scalar: bass.BassEngine):
            scalar.dma_start(out=wall[H:P, 0:2 * P], in_=w_d[H:P, 0:2 * P]
                             ).then_inc(dww, 16)
            scalar.dma_start(out=xin[0][H:P, 0:SP0], in_=x_d[H:P, 0:SP0]
                             ).then_inc(dxa0a, 16)
            scalar.dma_start(out=xin[0][H:P, SP0:C_LEN[0]],
                             in_=x_d[H:P, SP0:C_LEN[0]]).then_inc(dxa0, 16)
            x_dma(scalar, NA, 0, P, dxv0)       # xV0 whole
            x_dma(scalar, 1, H, P, dxa1)
            for s, (k, ci) in enumerate(a_stages):
                base = PSA_BASE[s % 2]
                cl = C_LEN[ci]
                if s == 0:
                    for (a, b_), key in (((0, SP0), "Aa"), ((SP0, cl),
                                                           "Ab")):
                        scalar.activation(
                            h_out(k, ci)[:, a:b_], ps[:, base + a:base + b_],
                            Tanh, bias=0.0, scale=1.0,
                        )._wait_ge(pe_sem, pe_pos[(key, 0)] + 1
                                   ).then_inc(act_sem, 1)
                elif s == NSA - 1:
                    for a, b_ in ((0, TH), (TH, cl)):
                        act = scalar.activation(
                            h_out(k, ci)[:, a:b_], ps[:, base + a:base + b_],
                            Tanh, bias=0.0, scale=1.0)
                        if a == 0:
                            act._wait_ge(pe_sem, pe_pos[("A", s)] + 1)
                        act.then_inc(act_sem, 1)
                else:
                    scalar.activation(
                        h_out(k, ci), ps[:, base:base + cl], Tanh,
                        bias=0.0, scale=1.0,
                    )._wait_ge(pe_sem, pe_pos[("A", s)] + 1
                               ).then_inc(act_sem, 1)
            # remaining output quarters on the now-idle ACT queue
            y_dma(scalar, 5, act_sem, a_cum[NSA - 1], H, P, TH, CAS[-1])
            y_dma(scalar, NA + 1, dve_sem, v_cum[NSV - 1], 0, P, VH, CV)

        @block.gpsimd
        def _(g: bass.BassEngine):
            # stay clear of the fill window; software-DGE transfers use
            # their own semaphores (dxg cumulative in queue order)
            g.wait_ge(dxa1, 32)
            x_dma(g, 2, H, P, dxg)
            x_dma(g, 3, H, P, dxg)
            x_dma(g, 4, H, P, dxg)
            x_dma(g, 5, H, P, dxg)
            y_dma(g, 0, act_sem, a_cum[idx_a[(5, 0)]], dsem=doutg)
            y_dma(g, 1, act_sem, a_cum[idx_a[(5, 1)]], dsem=doutg)
            y_dma(g, 2, act_sem, a_cum[idx_a[(5, 2)]], dsem=doutg)
            y_dma(g, NA, dve_sem, v_cum[idx_v[(5, 0)]], dsem=doutg)
            y_dma(g, 4, act_sem, a_cum[idx_a[(5, 4)]], H, P, dsem=doutg)
            y_dma(g, 5, act_sem, a_cum[NSA - 1] - 1, H, P, 0, TH,
                  dsem=doutg)
            y_dma(g, NA + 1, dve_sem, v_cum[NSV - 1] - 1, 0, P, 0, VH,
                  dsem=doutg)

        def _vector_body(v: bass.BassEngine):
            psv = ps[:, PSV_BASE:PSV_BASE + CV]
            xe, t, w, u1, u2, f, p, p2 = (vt[n] for n in
                                          ("xe", "t", "w", "u1", "u2",
                                           "f", "p", "p2"))
            for sv, (k, j) in enumerate(v_stages):
                v.wait_ge(pe_sem, pe_pos[("V", sv)] + 1)
                # evac + clamp from PSUM f32 -> SBUF fp16; within-engine
                # deps below ride program order (the DVE pipe drains
                # between ops), so only the evac and the final y inc
                # dve_sem for PE / DMA consumers
                v.tensor_scalar(xe, psv, PU, -PU, Alu.min, Alu.max
                                ).then_inc(dve_sem, 1)
                # factored deg-9: all TT (2x) / TS (4x); STT would
                # run at 1x (no 2x uop) so each (a+s)*b is TS+TT
                v.tensor_tensor(t, xe, xe, Alu.mult)
                v.tensor_scalar(w, t, PA1, None, Alu.add)
                v.tensor_tensor(u1, w, t, Alu.mult)
                v.tensor_scalar(w, t, PA2, None, Alu.add)
                v.tensor_tensor(u2, w, t, Alu.mult)
                v.tensor_scalar(f, u1, PB1, PC4, Alu.add, Alu.mult)
                v.tensor_scalar(w, u2, PB2, None, Alu.add)
                v.tensor_tensor(p2, w, f, Alu.mult)
                out = h_out(k, NA + j)
                if sv == NSV - 1:
                    v.tensor_tensor(out[:, 0:VH], p2[:, 0:VH], xe[:, 0:VH],
                                    Alu.mult).then_inc(dve_sem, 1)
                    v.tensor_tensor(out[:, VH:CV], p2[:, VH:CV],
                                    xe[:, VH:CV], Alu.mult
                                    ).then_inc(dve_sem, 1)
                else:
                    v.tensor_tensor(out, p2, xe, Alu.mult
                                    ).then_inc(dve_sem, 1)

        block.vector(_vector_body)

        @block.tensor
        def _(tensor: bass.BassEngine):
            tensor.wait_ge(dww, 32)
            seen1 = seen2 = False
            for lane, i, _t in pe_order:
                if lane in ("A", "Aa", "Ab"):
                    s = i
                    k, ci = a_stages[s]
                    base = PSA_BASE[s % 2]
                    coff, clen = A_OFF[ci], C_LEN[ci]
                    if lane == "Aa":
                        clen = SP0
                    elif lane == "Ab":
                        base, coff = base + SP0, coff + SP0
                        clen = C_LEN[ci] - SP0
                    rhs = h_in(k, ci)
                    if lane == "Ab":
                        rhs = rhs[:, SP0:]
                else:
                    sv = i
                    k, j = v_stages[sv]
                    base = PSV_BASE
                    coff, clen = V_OFF[j], CV
                    rhs = h_in(k, NA + j)
                ks = WSETS[k]
                if ks >= 1 and not seen1:
                    tensor.wait_ge(dww12, 16)   # set 1
                    seen1 = True
                if ks == 2 and not seen2:
                    tensor.wait_ge(dww12, 32)   # + set 2 (same queue)
                    seen2 = True
                if k == 0:
                    if lane != "V":
                        if lane == "Aa":
                            tensor.wait_ge(dxa0a, 32)
                        elif lane == "Ab":
                            tensor.wait_ge(dxa[0], 32)
                        elif ci < 2 and s > 0:
                            tensor.wait_ge(dxa[ci], 32)
                        elif ci >= 2:
                            # top half on sync; bottom halves ride the
                            # gpsimd software-DGE queue whose completion
                            # order is not guaranteed -> wait for all 4
                            # (they land ~16us; first needed ~26us)
                            tensor.wait_ge(dxa[ci], 16)
                            tensor.wait_ge(dxg, 64)
                    else:
                        tensor.wait_ge(dxv[j], 16)
                # folded input-ready + PSUM WAR wait
                wv = 0
                if lane in ("A", "Aa", "Ab"):
                    if s >= 2:
                        wv = a_cum[s - 2]
                    if k > 0:
                        wv = max(wv, a_cum[idx_a[(k - 1, ci)]])
                else:
                    if sv >= 1:
                        wv = (v_cum[sv - 2] + 1) if sv >= 2 else 1
                wins = _mm_windows(coff, clen, base)
                for wi, (lo, hi, zone) in enumerate(wins):
                    kz = ks * 2 + zone
                    mm = tensor.matmul(ps[:, base + lo:base + hi],
                                       wall[:, kz * P:(kz + 1) * P],
                                       rhs[:, lo:hi],
                                       start=True, stop=True)
                    if wi == 0 and wv > 0:
                        if lane == "V":
                            mm._wait_ge(dve_sem, wv)
                        else:
                            mm._wait_ge(act_sem, wv)
                mm.then_inc(pe_sem, 1)

    nc.compile()
    return nc


def _film_params(c, Wk, bk, Wsk, bsk, Wbk, bbk):
    """A[b] = diag(scale[b]) @ Wk ; d[b] = scale[b]*bk + shift[b], float64."""
    c = c.astype(np.float64)
    scale = 1.0 / (1.0 + np.exp(-(c @ Wsk.astype(np.float64).T
                                  + bsk.astype(np.float64))))     # [B,3]
    shift = c @ Wbk.astype(np.float64).T + bbk.astype(np.float64)  # [B,3]
    A = scale[:, :, None] * Wk.astype(np.float64)[None]            # [B,3,3]
    d = scale * bk.astype(np.float64) + shift                      # [B,3]
    return A, d


def kernel(t, x, c,
           W0, b0, Ws0, bs0, Wb0, bb0,
           W1, b1, Ws1, bs1, Wb1, bb1,
           W2, b2, Ws2, bs2, Wb2, bb2):
    global LAST_EXEC_NS
    if PROFILE:
        _install_profile_shim()
    from concourse.bass_utils import run_bass_kernel_spmd

    x = np.asarray(x)
    c = np.asarray(c)
    (W0, b0, Ws0, bs0, Wb0, bb0, W1, b1, Ws1, bs1, Wb1, bb1,
     W2, b2, Ws2, bs2, Wb2, bb2) = (
        np.asarray(a) for a in (W0, b0, Ws0, bs0, Wb0, bb0,
                                W1, b1, Ws1, bs1, Wb1, bb1,
                                W2, b2, Ws2, bs2, Wb2, bb2))
    out_dtype = x.dtype

    if "prog" not in _CACHE:
        _CACHE["prog"] = _build_program()
    nc = _CACHE["prog"]

    # ---- host: FiLM affine params per (weight-set, batch), float64 ----
    sets = [
        _film_params(c, W0, b0, Ws0, bs0, Wb0, bb0),
        _film_params(c, W1, b1, Ws1, bs1, Wb1, bb1),
        _film_params(c, W2, b2, Ws2, bs2, Wb2, bb2),
    ]

    # ---- host: shard + relayout x ----
    # [B, N, 3] -> per core [128, LC] fp16: stream t on partitions
    # 3t..3t+2, ones-row 126, zero-row 127. Stream table per core:
    # 10 full streams per batch + one shared boundary stream per batch
    # pair, switching batch at column BSPLIT.
    xp = np.ascontiguousarray(x, dtype=np.float32)
    xt = np.ascontiguousarray(xp.transpose(0, 2, 1))   # [B, 3, N]

    # (batch_lo, batch_hi, offset): full streams have lo == hi
    stream_table = []
    for pair in range(2):
        ba, bb = 2 * pair, 2 * pair + 1
        stream_table += [(ba, ba, t * LC) for t in range(10)]
        stream_table.append((ba, bb, NFULL))
        stream_table += [(bb, bb, t * LC) for t in range(10)]

    in_maps = []
    for cc in range(NCORES):
        b0 = cc * BPC
        X = np.zeros((P, LC), np.float16)
        for t, (blo, bhi, off) in enumerate(stream_table):
            for c_ in range(D):
                row = 3 * t + c_
                if blo == bhi:
                    X[row] = xt[b0 + blo, c_, off:off + LC]
                else:
                    X[row, :BSPLIT] = xt[b0 + blo, c_, NFULL:N]
                    X[row, BSPLIT:2 * BSPLIT] = xt[b0 + bhi, c_, NFULL:N]
        X[126] = 1.0                # ones-row: carries the bias via matmul
        W6 = np.zeros((P, 6 * P), np.float16)
        for k in range(3):
            A, dv = sets[k]
            for zone in range(2):
                c0 = (k * 2 + zone) * P
                for t, (blo, bhi, off) in enumerate(stream_table):
                    b = b0 + (blo if zone == 0 else bhi)
                    for ci_ in range(3):
                        for cj in range(3):
                            W6[3 * t + cj, c0 + 3 * t + ci_] = \
                                np.float16(A[b, ci_, cj])
                        # bias d rides the ones-row
                        W6[126, c0 + 3 * t + ci_] = np.float16(dv[b, ci_])
                # ones-row regenerates itself: tanh(16.0) == 1.0 in fp16,
                # and the DVE clamp U maps 16.0 -> exactly 1.0 too
                W6[126, c0 + 126] = np.float16(16.0)
        in_maps.append({"x": X, "w": W6})

    res = run_bass_kernel_spmd(nc, in_maps, list(range(NCORES)),
                               trace=bool(PROFILE))
    if PROFILE:
        LAST_EXEC_NS = res.exec_time_ns

    # ---- host: gather + inverse layout ----
    yt = np.empty((B, D, N), np.float32)
    for cc in range(NCORES):
        Y = res.results[cc]["y"]                       # [P, LC] fp16
        b0 = cc * BPC
        for t, (blo, bhi, off) in enumerate(stream_table):
            for c_ in range(D):
                row = 3 * t + c_
                if blo == bhi:
                    yt[b0 + blo, c_, off:off + LC] = Y[row]
                else:
                    yt[b0 + blo, c_, NFULL:N] = Y[row, :BSPLIT]
                    yt[b0 + bhi, c_, NFULL:N] = Y[row, BSPLIT:2 * BSPLIT]
    out = np.ascontiguousarray(yt.transpose(0, 2, 1)).astype(
        out_dtype, copy=False)
    return out
